# revision 50
# baseline (speedup 1.0000x reference)
"""Trainium2 Bass kernel for nn_CPLinear (CP-decomposed QKV projection with RoPE).

Computes, for x:(2,4096,2048) and CP-factor weights:
    A_t = x @ W_A_t  (per-token head coefficients),  B_t = x @ W_B_t (shared bases)
    q = einsum('bshr,bsrd->bshd', A_q, rope(B_q)) / 12
    k = A_k * rope(B_k)   (rank-1)
    v = A_v * B_v         (rank-1)

Strategy (8 cores, data-parallel over the 8192 tokens, 1024 tokens/core):
  - All 6 projections fused into one [2048 x 2016] bf16 matmul (PE), with the
    1/12 scale and (h,r)->(r,h) reorder folded into W_A_q host-side.
  - W/x are DMAed k-chunk-interleaved in need-order; tiles 0+1 run a k-outer
    PSUM-resident warmup so the PE starts ~2us into the load.
  - The projection runs as two 2-bank column passes (A/kv block + B_q lo,
    then B_q hi) rotating over three PSUM groups; two banks are dedicated to
    the block-diagonal q matmuls, so no PSUM slot reuse ever stalls the PE.
  - RoPE is applied to [B_q|B_k] as 13 rank slices in 4 wide DVE ops, using
    host-precomputed cos2=[cos,cos] / sinsw=[sin,-sin] tables (rotate-half
    folded into the sign).
  - The per-token rank-12 contraction for q runs on the PE as a block-diagonal
    matmul: 8 tokens/matmul, K=96=(8 tokens x 12 r), M=128=(8 tokens x 16 h),
    N=128=d. Operands are built by a DRAM bounce + per-slot scatter DMAs on
    the sync/gpsimd queues only (engine streams are in-order, so chain-
    dependent DMA issues must never sit on the eviction engine; the fat k/v
    store rides scalar's queue).
  - k/v are per-partition broadcast products (DVE tensor_tensor), fused into
    one store.
  - Outputs are written bf16 in on-chip layout; host reorders q and widens.

Note: this target power-throttles the PE (~29% of the run at 0.5 util), so
wall-clock plateaus near ~188us +-5 regardless of schedule micro-tuning; only
energy-level cuts (bytes moved, MAC-slots, precision) move it further.
"""

import sys

for _p in ("/opt/trn_rl_repo",):
    if _p not in sys.path:
        sys.path.insert(0, _p)

import numpy as np
import ml_dtypes

BF16 = ml_dtypes.bfloat16

SH = 1024          # tokens per core
H = 2048           # hidden
KT = H // 128      # 16 k-tiles
NT = SH // 128     # 8 token tiles per core
NOUT = 2016        # fused projection output width
NH, HD, RQ = 16, 128, 12

_CACHE = {}


def make_nc():
    import concourse.bacc as bacc
    from concourse import mybir

    dt = mybir.dt

    nc = bacc.Bacc(
        "TRN2",
        target_bir_lowering=False,
        debug=False,
        enable_asserts=False,
        num_devices=8,
    )

    x_d = nc.dram_tensor("x", (H, SH), dt.bfloat16, kind="ExternalInput")  # pre-transposed host-side
    w_d = nc.dram_tensor("w", (KT, 128, NOUT), dt.bfloat16, kind="ExternalInput")
    # cos2 = [cos, cos], sinsw = [+sin, -sin] (rotate-half folded into sign)
    cos_d = nc.dram_tensor("cos2", (SH, 128), dt.bfloat16, kind="ExternalInput")
    sin_d = nc.dram_tensor("sinsw", (SH, 128), dt.bfloat16, kind="ExternalInput")
    # q in on-chip layout: row (tile,t,h) = tile*128 + t*16 + h, col g*128+d
    q_d = nc.dram_tensor("q", (SH, NH * HD), dt.bfloat16, kind="ExternalOutput")
    # k and v fused into one store: cols 0:2048 = k, 2048:4096 = v
    kv_d = nc.dram_tensor(
        "kv", (SH, 2 * NH * HD), dt.bfloat16, kind="ExternalOutput"
    )
    return nc, (x_d, w_d, cos_d, sin_d, q_d, kv_d)


def build_body(nc, tc, tensors):
    from contextlib import ExitStack

    from concourse import mybir

    dt = mybir.dt
    x_d, w_d, cos_d, sin_d, q_d, kv_d = tensors

    with ExitStack() as ctx:
        P = ctx.enter_context
        const_pool = P(tc.tile_pool(name="const", bufs=1))
        w_sb = const_pool.tile([128, KT * NOUT], dt.bfloat16, tag="w_sb")
        cos_sb = const_pool.tile([128, NT * 128], dt.bfloat16, tag="cos_sb")
        sin_sb = const_pool.tile([128, NT * 128], dt.bfloat16, tag="sin_sb")
        xT = const_pool.tile([128, KT * SH], dt.bfloat16, tag="xT")
        # combined block-diagonal operand holders: cols 0:2048 = stacked roped
        # B_q (rhs), cols 2048:4096 = block-diag A' (lhsT), per 8-token slot.
        bd_bufs = [
            const_pool.tile([128, 4096], dt.bfloat16, tag=f"bd{i}", name=f"bd{i}")
            for i in range(3)
        ]

        # constant loads: cos/sin first (tiny), then W/x k-chunk interleaved
        # so the k-outer warmup on tiles 0/1 can start after the first chunk.
        w_v = w_sb[:].rearrange("p (k n) -> p k n", k=KT)
        wd_v = w_d[:].rearrange("k p n -> p k n")
        x_v = xT[:].rearrange("p (k t) -> p k t", k=KT)
        xd_v = x_d[:].rearrange("(k p) t -> p k t", p=128)
        # All load pieces are issued in strict need-order, round-robin across
        # the three DMA-capable engines: per-queue service is only a fraction
        # of aggregate HBM bandwidth when several queues are active, so the
        # earliest-needed pieces must not sit behind later ones in any queue.
        # Warmup x covers tokens 0:384 (tiles 0/1/2); the rest of x is first
        # needed by tile 3's projection, much later.
        nc.gpsimd.dma_start(
            out=cos_sb[:].rearrange("p (t n) -> p t n", t=NT),
            in_=cos_d[:].rearrange("(t p) n -> p t n", p=128),
        )
        nc.gpsimd.dma_start(
            out=sin_sb[:].rearrange("p (t n) -> p t n", t=NT),
            in_=sin_d[:].rearrange("(t p) n -> p t n", p=128),
        )
        for kk in range(KT):
            sl = slice(kk, kk + 1)
            nc.scalar.dma_start(out=w_v[:, sl], in_=wd_v[:, sl])
            nc.sync.dma_start(out=x_v[:, sl, 0:384], in_=xd_v[:, sl, 0:384])
        for j in range(8):
            sl = slice(2 * j, 2 * j + 2)
            nc.sync.dma_start(out=x_v[:, sl, 384:SH], in_=xd_v[:, sl, 384:SH])
        # zero the A halves once (block-diag zeros outside the scatter
        # positions); the B halves are fully rewritten by the readbacks.
        for tl in bd_bufs:
            nc.gpsimd.memset(tl[0:96, 2048:4096], 0.0)

        # PSUM: one pool of 8 bank-sized [128,512] slots, explicitly
        # assigned. s0..s5 form three 2-bank groups rotated by the two-pass
        # projection (pass A = fused cols 0:992 = [A/kv block | B_q 0:512],
        # pass B = cols 992:2016 = B_q 512:1536); s6/s7 are dedicated to
        # consume's block-diag q matmuls. No slot is ever reused by the next
        # tile before its eviction has long finished.
        ps_pool = P(tc.tile_pool(name="ps", bufs=1, space="PSUM"))
        bq_pool = P(tc.tile_pool(name="bq", bufs=3))
        bqr_pool = P(tc.tile_pool(name="bqr", bufs=3))
        tmp_pool = P(tc.tile_pool(name="tmp", bufs=3))
        small_pool = P(tc.tile_pool(name="small", bufs=4))
        out_pool = P(tc.tile_pool(name="outs", bufs=3))
        dram_pool = P(tc.tile_pool(name="scr", bufs=3, space="DRAM"))

        state = {}

        def ps_tile(slot, nm):
            return ps_pool.tile(
                [128, 512], dt.float32, tag=f"s{slot}", name=nm
            )

        def mm_pass(it, ps2, half, kk, start, stop):
            t0 = it * 128
            lh = xT[:, kk * SH + t0 : kk * SH + t0 + 128]
            wb = kk * NOUT + (0 if half == 0 else 992)
            w0 = 480 if half == 0 else 512
            nc.tensor.matmul(
                ps2[0][:, 0:w0], lh, w_sb[:, wb : wb + w0],
                start=start, stop=stop,
            )
            nc.tensor.matmul(
                ps2[1][:], lh, w_sb[:, wb + w0 : wb + w0 + 512],
                start=start, stop=stop,
            )

        def alloc_tiles(it):
            ak_sb = small_pool.tile([128, 16], dt.bfloat16, tag="ak_sb")
            av_sb = small_pool.tile([128, 16], dt.bfloat16, tag="av_sb")
            bv_sb = small_pool.tile([128, 128], dt.bfloat16, tag="bv_sb")
            # bq_sb holds raw [B_q (12x128) | B_k (128)] = 13 rank slices
            bq_sb = bq_pool.tile([128, 1664], dt.bfloat16, tag="bq_sb")
            # bqr holds roped [B_q|B_k] (0:1664) and A' (1664:1856) so the
            # DRAM bounce is a single DMA
            bqr = bqr_pool.tile([128, 1856], dt.bfloat16, tag="bqr_t")
            return ak_sb, av_sb, bv_sb, bq_sb, bqr

        def evict_A(it, psA, tiles):
            """A/kv-block + first B_q chunk evictions (run during pass B)."""
            ak_sb, av_sb, bv_sb, bq_sb, bqr = tiles
            nc.scalar.copy(bq_sb[:, 0:512], psA[1][:])
            nc.vector.tensor_copy(bq_sb[:, 1536:1664], psA[0][:, 224:352])
            nc.scalar.copy(bqr[:, 1664:1856], psA[0][:, 0:192])
            nc.scalar.copy(ak_sb[:], psA[0][:, 192:208])
            nc.scalar.copy(av_sb[:], psA[0][:, 208:224])
            nc.scalar.copy(bv_sb[:], psA[0][:, 352:480])

        def evict_B_and_post(it, psB, tiles):
            """pass-B evictions, RoPE, bounce + scatter, k/v for tile it."""
            ak_sb, av_sb, bv_sb, bq_sb, bqr = tiles
            t0 = it * 128
            bd = bd_bufs[it % 3]

            nc.scalar.copy(bq_sb[:, 512:1024], psB[0][:])
            nc.vector.tensor_copy(bq_sb[:, 1024:1536], psB[1][:])

            # ---- RoPE on [B_q|B_k] as 13 rank slices (DVE, 4 wide ops) ----
            # t = in*[cos,cos]; u = in*[+sin,-sin];
            # out_lo = t_lo + u_hi; out_hi = t_hi + u_lo
            t_a = tmp_pool.tile([128, 1664], dt.bfloat16, tag="t_a")
            t_b = tmp_pool.tile([128, 1664], dt.bfloat16, tag="t_b")
            R13 = RQ + 1
            cos_t = (
                cos_sb[:, it * 128 : (it + 1) * 128]
                .unsqueeze(1)
                .broadcast_to([128, R13, 128])
            )
            sin_t = (
                sin_sb[:, it * 128 : (it + 1) * 128]
                .unsqueeze(1)
                .broadcast_to([128, R13, 128])
            )
            bqv = bq_sb[:].rearrange("p (r c) -> p r c", r=R13)
            nc.vector.tensor_mul(
                t_a[:].rearrange("p (r c) -> p r c", r=R13), bqv, cos_t
            )
            nc.vector.tensor_mul(
                t_b[:].rearrange("p (r c) -> p r c", r=R13), bqv, sin_t
            )
            tav = t_a[:].rearrange("p (r two d) -> p r two d", r=R13, two=2)
            tbv = t_b[:].rearrange("p (r two d) -> p r two d", r=R13, two=2)
            bqrv = bqr[:, 0:1664].rearrange(
                "p (r two d) -> p r two d", r=R13, two=2
            )
            nc.vector.tensor_add(bqrv[:, :, 0], tav[:, :, 0], tbv[:, :, 1])
            nc.vector.tensor_add(bqrv[:, :, 1], tav[:, :, 1], tbv[:, :, 0])
            bkr_sb = bqr[:, 1536:1664]

            # ---- bounce bqr to DRAM, then scatter back into bd ----
            scr = dram_pool.tile([128, 1856], dt.bfloat16, tag="scr_b")
            nc.sync.dma_start(out=scr[:], in_=bqr[:])

            # ---- k, v (DVE tensor_mul) + fused output DMA ----
            kvsb = out_pool.tile([128, 4096], dt.bfloat16, tag="kvsb")
            nc.vector.tensor_mul(
                kvsb[:, 0:2048].rearrange("p (h d) -> p h d", h=NH),
                bkr_sb.unsqueeze(1).broadcast_to([128, NH, 128]),
                ak_sb[:].unsqueeze(2).broadcast_to([128, NH, 128]),
            )
            nc.vector.tensor_mul(
                kvsb[:, 2048:4096].rearrange("p (h d) -> p h d", h=NH),
                bv_sb[:].unsqueeze(1).broadcast_to([128, NH, 128]),
                av_sb[:].unsqueeze(2).broadcast_to([128, NH, 128]),
            )
            nc.scalar.dma_start(out=kv_d[t0 : t0 + 128, :], in_=kvsb[:])

            # scatter readbacks: B rhs into bd[:, 0:2048], A lhsT (block-diag)
            # into bd[:, 2048:4096]. Chain-critical, so they live only on the
            # sync/gpsimd queues — scalar's engine stream (evictions) must
            # never block on the bounce, and scalar's HW queue carries the
            # fat kv store.
            sa_v = scr[:, 1664:1856].rearrange(
                "(g t) (r h) -> t r g h", t=8, r=RQ
            )
            sb_v = scr[:, 0:1536].rearrange("(g t) (r d) -> t r g d", t=8, r=RQ)
            l_v = bd[0:96, 2048:4096].rearrange(
                "(t r) (g c) -> t r g c", t=8, g=16
            )
            d_v = bd[0:96, 0:2048].rearrange("(t r) (g d) -> t r g d", t=8, g=16)
            engs = (nc.gpsimd, nc.sync)
            for t in range(8):
                engs[t % 2].dma_start(
                    out=l_v[t][:, :, t * 16 : (t + 1) * 16], in_=sa_v[t]
                )
                engs[(t + 1) % 2].dma_start(out=d_v[t], in_=sb_v[t])

            state[it] = bd

        def consume(it):
            """q contraction (slots s6/s7) + output DMA for tile it."""
            t0 = it * 128
            bd = state.pop(it)

            qsb = out_pool.tile([128, 2048], dt.bfloat16, tag="qsb")
            for gq in range(4):
                qp = ps_tile(6 + gq % 2, f"qp{it}_{gq}")
                for j in range(4):
                    g = gq * 4 + j
                    nc.tensor.matmul(
                        qp[:, j * 128 : (j + 1) * 128],
                        bd[0:96, 2048 + g * 128 : 2048 + (g + 1) * 128],
                        bd[0:96, g * 128 : (g + 1) * 128],
                        start=True,
                        stop=True,
                    )
                if gq % 2 == 0:
                    nc.vector.tensor_copy(
                        qsb[:, gq * 512 : (gq + 1) * 512], qp[:]
                    )
                else:
                    nc.scalar.copy(qsb[:, gq * 512 : (gq + 1) * 512], qp[:])

            # dense on-chip-layout store; host reorders (t,h)(g,d)->(g,t)(h,d)
            nc.sync.dma_start(out=q_d[t0 : t0 + 128, :], in_=qsb[:])

        # ---- schedule ----
        # warmup: tiles 0 and 1 k-outer across all 8 slots, so the PE starts
        # as soon as the first W/x k-chunk lands and never waits on the load.
        wt0 = alloc_tiles(0)
        wt1 = alloc_tiles(1)
        wA0 = [ps_tile(0, "wA0_0"), ps_tile(1, "wA0_1")]
        wB0 = [ps_tile(2, "wB0_0"), ps_tile(3, "wB0_1")]
        wA1 = [ps_tile(4, "wA1_0"), ps_tile(5, "wA1_1")]
        wB1 = [ps_tile(6, "wB1_0"), ps_tile(7, "wB1_1")]
        for kk in range(KT):
            st, sp = kk == 0, kk == KT - 1
            mm_pass(0, wA0, 0, kk, st, sp)
            mm_pass(0, wB0, 1, kk, st, sp)
            mm_pass(1, wA1, 0, kk, st, sp)
            mm_pass(1, wB1, 1, kk, st, sp)
        evict_A(0, wA0, wt0)
        evict_B_and_post(0, wB0, wt0)
        evict_A(1, wA1, wt1)
        evict_B_and_post(1, wB1, wt1)

        GRP = ((0, 1), (2, 3), (4, 5))

        def produce(it):
            tiles = alloc_tiles(it)
            seq = 2 * (it - 2)
            gA, gB = GRP[seq % 3], GRP[(seq + 1) % 3]
            psA = [ps_tile(gA[0], f"pA{it}_0"), ps_tile(gA[1], f"pA{it}_1")]
            for kk in range(KT):
                mm_pass(it, psA, 0, kk, kk == 0, kk == KT - 1)
            evict_A(it, psA, tiles)
            psB = [ps_tile(gB[0], f"pB{it}_0"), ps_tile(gB[1], f"pB{it}_1")]
            for kk in range(KT):
                mm_pass(it, psB, 1, kk, kk == 0, kk == KT - 1)
            evict_B_and_post(it, psB, tiles)

        # consume(i) is emitted after produce(i+2): its instructions land ~two
        # tiles after its data is ready, so no engine stream ever blocks.
        produce(2)
        consume(0)
        produce(3)
        consume(1)
        produce(4)
        consume(2)
        produce(5)
        consume(3)
        produce(6)
        consume(4)
        produce(7)
        consume(5)
        consume(6)
        consume(7)


def build_program():
    import concourse.tile as tile

    nc, tensors = make_nc()
    with tile.TileContext(nc) as tc:
        build_body(nc, tc, tensors)
    nc.compile()
    return nc


def _get_program():
    if "nc" not in _CACHE:
        _CACHE["nc"] = build_program()
    return _CACHE["nc"]


def make_in_maps(x, W_A_q, W_B_q, W_A_k, W_B_k, W_A_v, W_B_v):
    """Shard + preprocess full inputs into per-core input maps."""
    x = np.asarray(x)
    B, S, Hh = x.shape
    x2 = np.ascontiguousarray(x.reshape(B * S, Hh))

    # fold the 1/RQ scale and the (h,r)->(r,h) column reorder into W_A_q
    WAq = np.asarray(W_A_q).reshape(Hh, NH, RQ).transpose(0, 2, 1).reshape(
        Hh, NH * RQ
    ) / np.float32(RQ)
    Wall = np.concatenate(
        [
            WAq,
            np.asarray(W_A_k),
            np.asarray(W_A_v),
            np.asarray(W_B_k),
            np.asarray(W_B_v),
            np.asarray(W_B_q),
        ],
        axis=1,
    )
    assert Wall.shape == (Hh, NOUT)
    Wt = np.ascontiguousarray(Wall.reshape(KT, 128, NOUT)).astype(BF16)

    inv = 1.0 / (10000.0 ** (np.arange(0, HD, 2, dtype=np.float32) / HD))
    ang = np.arange(S, dtype=np.float32)[:, None] * inv[None, :]
    c, s = np.cos(ang), np.sin(ang)
    cos2 = np.ascontiguousarray(np.concatenate([c, c], axis=1)).astype(BF16)
    sinsw = np.ascontiguousarray(np.concatenate([s, -s], axis=1)).astype(BF16)

    in_maps = []
    for i in range(8):
        tok0 = i * SH
        pos = np.arange(tok0, tok0 + SH) % S
        in_maps.append(
            {
                # pre-transposed (hidden, tokens) so on-chip loads are plain
                "x": np.ascontiguousarray(x2[tok0 : tok0 + SH].T).astype(BF16),
                "w": Wt,
                "cos2": np.ascontiguousarray(cos2[pos]),
                "sinsw": np.ascontiguousarray(sinsw[pos]),
            }
        )
    return in_maps, (B, S)


def assemble_outputs(results, B, S):
    # q rows are (tile, t, h) with token = tile*128 + g*8 + t, cols (g, d)
    qs = []
    for i in range(8):
        a = results[i]["q"].astype(np.float32)
        a = a.reshape(NT, 8, 16, 16, 128).transpose(0, 3, 1, 2, 4)
        qs.append(a.reshape(SH, NH, HD))
    q = np.concatenate(qs, axis=0).reshape(B, S, NH, HD)
    k = np.concatenate(
        [results[i]["kv"][:, 0:2048].astype(np.float32) for i in range(8)],
        axis=0,
    ).reshape(B, S, NH, HD)
    v = np.concatenate(
        [results[i]["kv"][:, 2048:4096].astype(np.float32) for i in range(8)],
        axis=0,
    ).reshape(B, S, NH, HD)
    return q, k, v


def kernel(x, W_A_q, W_B_q, W_A_k, W_B_k, W_A_v, W_B_v):
    from concourse.bass_utils import run_bass_kernel_spmd

    nc = _get_program()
    in_maps, (B, S) = make_in_maps(x, W_A_q, W_B_q, W_A_k, W_B_k, W_A_v, W_B_v)
    res = run_bass_kernel_spmd(nc, in_maps, list(range(8))).results
    return assemble_outputs(res, B, S)


# revision 52
# speedup vs baseline: 1.0328x; 1.0328x over previous
"""Trainium2 Bass kernel for nn_CPLinear (CP-decomposed QKV projection with RoPE).

Computes, for x:(2,4096,2048) and CP-factor weights:
    A_t = x @ W_A_t  (per-token head coefficients),  B_t = x @ W_B_t (shared bases)
    q = einsum('bshr,bsrd->bshd', A_q, rope(B_q)) / 12
    k = A_k * rope(B_k)   (rank-1)
    v = A_v * B_v         (rank-1)

Strategy (8 cores, data-parallel over the 8192 tokens, 1024 tokens/core):
  - All 6 projections fused into one [2048 x 2016] bf16 matmul (PE), with the
    1/12 scale and (h,r)->(r,h) reorder folded into W_A_q host-side.
  - W/x are DMAed k-chunk-interleaved in need-order; tiles 0+1 run a k-outer
    PSUM-resident warmup so the PE starts ~2us into the load.
  - The projection runs as two 2-bank column passes (A/kv block + B_q lo,
    then B_q hi) rotating over three PSUM groups; two banks are dedicated to
    the block-diagonal q matmuls, so no PSUM slot reuse ever stalls the PE.
  - RoPE is applied to [B_q|B_k] as 13 rank slices in 4 wide DVE ops, using
    host-precomputed cos2=[cos,cos] / sinsw=[sin,-sin] tables (rotate-half
    folded into the sign).
  - The per-token rank-12 contraction for q runs on the PE as a block-diagonal
    matmul: 8 tokens/matmul, K=96=(8 tokens x 12 r), M=128=(8 tokens x 16 h),
    N=128=d. Operands are built by a DRAM bounce + per-slot scatter DMAs on
    the sync/gpsimd queues only (engine streams are in-order, so chain-
    dependent DMA issues must never sit on the eviction engine; the fat k/v
    store rides scalar's queue).
  - k/v are per-partition broadcast products (DVE tensor_tensor), fused into
    one store.
  - Outputs are written bf16 in on-chip layout; host reorders q and widens.

Note: this target power-throttles the PE (~29% of the run at 0.5 util), so
wall-clock plateaus near ~188us +-5 regardless of schedule micro-tuning; only
energy-level cuts (bytes moved, MAC-slots, precision) move it further.
"""

import sys

for _p in ("/opt/trn_rl_repo",):
    if _p not in sys.path:
        sys.path.insert(0, _p)

import numpy as np
import ml_dtypes

BF16 = ml_dtypes.bfloat16

SH = 1024          # tokens per core
H = 2048           # hidden
KT = H // 128      # 16 k-tiles
NT = SH // 128     # 8 token tiles per core
NOUT = 2016        # fused projection output width
NH, HD, RQ = 16, 128, 12

_CACHE = {}


def make_nc():
    import concourse.bacc as bacc
    from concourse import mybir

    dt = mybir.dt

    nc = bacc.Bacc(
        "TRN2",
        target_bir_lowering=False,
        debug=False,
        enable_asserts=False,
        num_devices=8,
    )

    x_d = nc.dram_tensor("x", (H, SH), dt.bfloat16, kind="ExternalInput")  # pre-transposed host-side
    w_d = nc.dram_tensor("w", (KT, 128, NOUT), dt.bfloat16, kind="ExternalInput")
    # cos2 = [cos, cos], sinsw = [+sin, -sin] (rotate-half folded into sign)
    cos_d = nc.dram_tensor("cos2", (SH, 128), dt.bfloat16, kind="ExternalInput")
    sin_d = nc.dram_tensor("sinsw", (SH, 128), dt.bfloat16, kind="ExternalInput")
    # q in on-chip layout: row (tile,t,h) = tile*128 + t*16 + h, col g*128+d
    q_d = nc.dram_tensor("q", (SH, NH * HD), dt.bfloat16, kind="ExternalOutput")
    # k and v fused into one store: cols 0:2048 = k, 2048:4096 = v
    kv_d = nc.dram_tensor(
        "kv", (SH, 2 * NH * HD), dt.bfloat16, kind="ExternalOutput"
    )
    return nc, (x_d, w_d, cos_d, sin_d, q_d, kv_d)


def build_body(nc, tc, tensors):
    from contextlib import ExitStack

    from concourse import mybir

    dt = mybir.dt
    x_d, w_d, cos_d, sin_d, q_d, kv_d = tensors

    with ExitStack() as ctx:
        P = ctx.enter_context
        const_pool = P(tc.tile_pool(name="const", bufs=1))
        w_sb = const_pool.tile([128, KT * NOUT], dt.bfloat16, tag="w_sb")
        cos_sb = const_pool.tile([128, NT * 128], dt.bfloat16, tag="cos_sb")
        sin_sb = const_pool.tile([128, NT * 128], dt.bfloat16, tag="sin_sb")
        xT = const_pool.tile([128, KT * SH], dt.bfloat16, tag="xT")
        # combined block-diagonal operand holders: cols 0:2048 = stacked roped
        # B_q (rhs), cols 2048:4096 = block-diag A' (lhsT), per 8-token slot.
        bd_bufs = [
            const_pool.tile([128, 4096], dt.bfloat16, tag=f"bd{i}", name=f"bd{i}")
            for i in range(3)
        ]

        # constant loads: cos/sin first (tiny), then W/x k-chunk interleaved
        # so the k-outer warmup on tiles 0/1 can start after the first chunk.
        w_v = w_sb[:].rearrange("p (k n) -> p k n", k=KT)
        wd_v = w_d[:].rearrange("k p n -> p k n")
        x_v = xT[:].rearrange("p (k t) -> p k t", k=KT)
        xd_v = x_d[:].rearrange("(k p) t -> p k t", p=128)
        # All load pieces are issued in strict need-order, round-robin across
        # the three DMA-capable engines: per-queue service is only a fraction
        # of aggregate HBM bandwidth when several queues are active, so the
        # earliest-needed pieces must not sit behind later ones in any queue.
        # Warmup x covers tokens 0:384 (tiles 0/1/2); the rest of x is first
        # needed by tile 3's projection, much later.
        # each W chunk is striped across the scalar+gpsimd queues (a single
        # queue's service rate is below the warmup's throttled PE demand);
        # cos/sin follow the chunk stream (first needed by tile 0's rope).
        for kk in range(KT):
            sl = slice(kk, kk + 1)
            nc.scalar.dma_start(
                out=w_v[:, sl, 0:1008], in_=wd_v[:, sl, 0:1008]
            )
            nc.gpsimd.dma_start(
                out=w_v[:, sl, 1008:NOUT], in_=wd_v[:, sl, 1008:NOUT]
            )
            nc.sync.dma_start(out=x_v[:, sl, 0:384], in_=xd_v[:, sl, 0:384])
        nc.scalar.dma_start(
            out=cos_sb[:].rearrange("p (t n) -> p t n", t=NT),
            in_=cos_d[:].rearrange("(t p) n -> p t n", p=128),
        )
        nc.gpsimd.dma_start(
            out=sin_sb[:].rearrange("p (t n) -> p t n", t=NT),
            in_=sin_d[:].rearrange("(t p) n -> p t n", p=128),
        )
        for j in range(8):
            sl = slice(2 * j, 2 * j + 2)
            nc.sync.dma_start(out=x_v[:, sl, 384:SH], in_=xd_v[:, sl, 384:SH])
        # zero the A halves once (block-diag zeros outside the scatter
        # positions); the B halves are fully rewritten by the readbacks.
        for tl in bd_bufs:
            nc.gpsimd.memset(tl[0:96, 2048:4096], 0.0)

        # PSUM: one pool of 8 bank-sized [128,512] slots, explicitly
        # assigned. s0..s5 form three 2-bank groups rotated by the two-pass
        # projection (pass A = fused cols 0:992 = [A/kv block | B_q 0:512],
        # pass B = cols 992:2016 = B_q 512:1536); s6/s7 are dedicated to
        # consume's block-diag q matmuls. No slot is ever reused by the next
        # tile before its eviction has long finished.
        ps_pool = P(tc.tile_pool(name="ps", bufs=1, space="PSUM"))
        bq_pool = P(tc.tile_pool(name="bq", bufs=3))
        bqr_pool = P(tc.tile_pool(name="bqr", bufs=3))
        tmp_pool = P(tc.tile_pool(name="tmp", bufs=3))
        small_pool = P(tc.tile_pool(name="small", bufs=4))
        out_pool = P(tc.tile_pool(name="outs", bufs=3))
        dram_pool = P(tc.tile_pool(name="scr", bufs=3, space="DRAM"))

        state = {}

        def ps_tile(slot, nm):
            return ps_pool.tile(
                [128, 512], dt.float32, tag=f"s{slot}", name=nm
            )

        def mm_pass(it, ps2, half, kk, start, stop):
            t0 = it * 128
            lh = xT[:, kk * SH + t0 : kk * SH + t0 + 128]
            wb = kk * NOUT + (0 if half == 0 else 992)
            w0 = 480 if half == 0 else 512
            nc.tensor.matmul(
                ps2[0][:, 0:w0], lh, w_sb[:, wb : wb + w0],
                start=start, stop=stop,
            )
            nc.tensor.matmul(
                ps2[1][:], lh, w_sb[:, wb + w0 : wb + w0 + 512],
                start=start, stop=stop,
            )

        def alloc_tiles(it):
            ak_sb = small_pool.tile([128, 16], dt.bfloat16, tag="ak_sb")
            av_sb = small_pool.tile([128, 16], dt.bfloat16, tag="av_sb")
            bv_sb = small_pool.tile([128, 128], dt.bfloat16, tag="bv_sb")
            # bq_sb holds raw [B_q (12x128) | B_k (128)] = 13 rank slices
            bq_sb = bq_pool.tile([128, 1664], dt.bfloat16, tag="bq_sb")
            # bqr holds roped [B_q|B_k] (0:1664) and A' (1664:1856) so the
            # DRAM bounce is a single DMA
            bqr = bqr_pool.tile([128, 1856], dt.bfloat16, tag="bqr_t")
            return ak_sb, av_sb, bv_sb, bq_sb, bqr

        def evict_A(it, psA, tiles):
            """A/kv-block + first B_q chunk evictions (run during pass B)."""
            ak_sb, av_sb, bv_sb, bq_sb, bqr = tiles
            nc.scalar.copy(bq_sb[:, 0:512], psA[1][:])
            nc.vector.tensor_copy(bq_sb[:, 1536:1664], psA[0][:, 224:352])
            nc.scalar.copy(bqr[:, 1664:1856], psA[0][:, 0:192])
            nc.scalar.copy(ak_sb[:], psA[0][:, 192:208])
            nc.scalar.copy(av_sb[:], psA[0][:, 208:224])
            nc.scalar.copy(bv_sb[:], psA[0][:, 352:480])

        def evict_B_and_post(it, psB, tiles):
            """pass-B evictions, RoPE, bounce + scatter, k/v for tile it."""
            ak_sb, av_sb, bv_sb, bq_sb, bqr = tiles
            t0 = it * 128
            bd = bd_bufs[it % 3]

            nc.scalar.copy(bq_sb[:, 512:1024], psB[0][:])
            nc.vector.tensor_copy(bq_sb[:, 1024:1536], psB[1][:])

            # ---- RoPE on [B_q|B_k] as 13 rank slices (DVE, 4 wide ops) ----
            # t = in*[cos,cos]; u = in*[+sin,-sin];
            # out_lo = t_lo + u_hi; out_hi = t_hi + u_lo
            t_a = tmp_pool.tile([128, 1664], dt.bfloat16, tag="t_a")
            t_b = tmp_pool.tile([128, 1664], dt.bfloat16, tag="t_b")
            R13 = RQ + 1
            cos_t = (
                cos_sb[:, it * 128 : (it + 1) * 128]
                .unsqueeze(1)
                .broadcast_to([128, R13, 128])
            )
            sin_t = (
                sin_sb[:, it * 128 : (it + 1) * 128]
                .unsqueeze(1)
                .broadcast_to([128, R13, 128])
            )
            bqv = bq_sb[:].rearrange("p (r c) -> p r c", r=R13)
            nc.vector.tensor_mul(
                t_a[:].rearrange("p (r c) -> p r c", r=R13), bqv, cos_t
            )
            nc.vector.tensor_mul(
                t_b[:].rearrange("p (r c) -> p r c", r=R13), bqv, sin_t
            )
            tav = t_a[:].rearrange("p (r two d) -> p r two d", r=R13, two=2)
            tbv = t_b[:].rearrange("p (r two d) -> p r two d", r=R13, two=2)
            bqrv = bqr[:, 0:1664].rearrange(
                "p (r two d) -> p r two d", r=R13, two=2
            )
            nc.vector.tensor_add(bqrv[:, :, 0], tav[:, :, 0], tbv[:, :, 1])
            nc.vector.tensor_add(bqrv[:, :, 1], tav[:, :, 1], tbv[:, :, 0])
            bkr_sb = bqr[:, 1536:1664]

            # ---- bounce bqr to DRAM (split across two queues so the
            # chain-critical transfer halves its latency), then scatter back
            scr = dram_pool.tile([128, 1856], dt.bfloat16, tag="scr_b")
            nc.sync.dma_start(out=scr[0:64, :], in_=bqr[0:64, :])
            nc.gpsimd.dma_start(out=scr[64:128, :], in_=bqr[64:128, :])

            # ---- k, v (DVE tensor_mul) + fused output DMA ----
            kvsb = out_pool.tile([128, 4096], dt.bfloat16, tag="kvsb")
            nc.vector.tensor_mul(
                kvsb[:, 0:2048].rearrange("p (h d) -> p h d", h=NH),
                bkr_sb.unsqueeze(1).broadcast_to([128, NH, 128]),
                ak_sb[:].unsqueeze(2).broadcast_to([128, NH, 128]),
            )
            nc.vector.tensor_mul(
                kvsb[:, 2048:4096].rearrange("p (h d) -> p h d", h=NH),
                bv_sb[:].unsqueeze(1).broadcast_to([128, NH, 128]),
                av_sb[:].unsqueeze(2).broadcast_to([128, NH, 128]),
            )
            nc.scalar.dma_start(out=kv_d[t0 : t0 + 128, :], in_=kvsb[:])

            # scatter readbacks: B rhs into bd[:, 0:2048], A lhsT (block-diag)
            # into bd[:, 2048:4096]. Chain-critical, so they live only on the
            # sync/gpsimd queues — scalar's engine stream (evictions) must
            # never block on the bounce, and scalar's HW queue carries the
            # fat kv store.
            sa_v = scr[:, 1664:1856].rearrange(
                "(g t) (r h) -> t r g h", t=8, r=RQ
            )
            sb_v = scr[:, 0:1536].rearrange("(g t) (r d) -> t r g d", t=8, r=RQ)
            l_v = bd[0:96, 2048:4096].rearrange(
                "(t r) (g c) -> t r g c", t=8, g=16
            )
            d_v = bd[0:96, 0:2048].rearrange("(t r) (g d) -> t r g d", t=8, g=16)
            engs = (nc.gpsimd, nc.sync)
            for t in range(8):
                engs[t % 2].dma_start(
                    out=l_v[t][:, :, t * 16 : (t + 1) * 16], in_=sa_v[t]
                )
                engs[(t + 1) % 2].dma_start(out=d_v[t], in_=sb_v[t])

            state[it] = bd

        def consume(it):
            """q contraction (slots s6/s7) + output DMA for tile it."""
            t0 = it * 128
            bd = state.pop(it)

            qsb = out_pool.tile([128, 2048], dt.bfloat16, tag="qsb")
            for gq in range(4):
                qp = ps_tile(6 + gq % 2, f"qp{it}_{gq}")
                for j in range(4):
                    g = gq * 4 + j
                    nc.tensor.matmul(
                        qp[:, j * 128 : (j + 1) * 128],
                        bd[0:96, 2048 + g * 128 : 2048 + (g + 1) * 128],
                        bd[0:96, g * 128 : (g + 1) * 128],
                        start=True,
                        stop=True,
                    )
                if gq % 2 == 0:
                    nc.vector.tensor_copy(
                        qsb[:, gq * 512 : (gq + 1) * 512], qp[:]
                    )
                else:
                    nc.scalar.copy(qsb[:, gq * 512 : (gq + 1) * 512], qp[:])

            # dense on-chip-layout store; host reorders (t,h)(g,d)->(g,t)(h,d)
            nc.sync.dma_start(out=q_d[t0 : t0 + 128, :], in_=qsb[:])

        # ---- schedule ----
        # warmup: tiles 0 and 1 k-outer across all 8 slots, so the PE starts
        # as soon as the first W/x k-chunk lands and never waits on the load.
        wt0 = alloc_tiles(0)
        wt1 = alloc_tiles(1)
        wA0 = [ps_tile(0, "wA0_0"), ps_tile(1, "wA0_1")]
        wB0 = [ps_tile(2, "wB0_0"), ps_tile(3, "wB0_1")]
        wA1 = [ps_tile(4, "wA1_0"), ps_tile(5, "wA1_1")]
        wB1 = [ps_tile(6, "wB1_0"), ps_tile(7, "wB1_1")]
        for kk in range(KT):
            st, sp = kk == 0, kk == KT - 1
            mm_pass(0, wA0, 0, kk, st, sp)
            mm_pass(0, wB0, 1, kk, st, sp)
            mm_pass(1, wA1, 0, kk, st, sp)
            mm_pass(1, wB1, 1, kk, st, sp)
        evict_A(0, wA0, wt0)
        evict_B_and_post(0, wB0, wt0)
        evict_A(1, wA1, wt1)
        evict_B_and_post(1, wB1, wt1)

        GRP = ((0, 1), (2, 3), (4, 5))

        def produce(it):
            tiles = alloc_tiles(it)
            seq = 2 * (it - 2)
            gA, gB = GRP[seq % 3], GRP[(seq + 1) % 3]
            psA = [ps_tile(gA[0], f"pA{it}_0"), ps_tile(gA[1], f"pA{it}_1")]
            for kk in range(KT):
                mm_pass(it, psA, 0, kk, kk == 0, kk == KT - 1)
            evict_A(it, psA, tiles)
            psB = [ps_tile(gB[0], f"pB{it}_0"), ps_tile(gB[1], f"pB{it}_1")]
            for kk in range(KT):
                mm_pass(it, psB, 1, kk, kk == 0, kk == KT - 1)
            evict_B_and_post(it, psB, tiles)

        # consume(i) is emitted after produce(i+2): its instructions land ~two
        # tiles after its data is ready, so no engine stream ever blocks.
        produce(2)
        consume(0)
        produce(3)
        consume(1)
        produce(4)
        consume(2)
        produce(5)
        consume(3)
        produce(6)
        consume(4)
        produce(7)
        consume(5)
        consume(6)
        consume(7)


def build_program():
    import concourse.tile as tile

    nc, tensors = make_nc()
    with tile.TileContext(nc) as tc:
        build_body(nc, tc, tensors)
    nc.compile()
    return nc


def _get_program():
    if "nc" not in _CACHE:
        _CACHE["nc"] = build_program()
    return _CACHE["nc"]


def make_in_maps(x, W_A_q, W_B_q, W_A_k, W_B_k, W_A_v, W_B_v):
    """Shard + preprocess full inputs into per-core input maps."""
    x = np.asarray(x)
    B, S, Hh = x.shape
    x2 = np.ascontiguousarray(x.reshape(B * S, Hh))

    # fold the 1/RQ scale and the (h,r)->(r,h) column reorder into W_A_q
    WAq = np.asarray(W_A_q).reshape(Hh, NH, RQ).transpose(0, 2, 1).reshape(
        Hh, NH * RQ
    ) / np.float32(RQ)
    Wall = np.concatenate(
        [
            WAq,
            np.asarray(W_A_k),
            np.asarray(W_A_v),
            np.asarray(W_B_k),
            np.asarray(W_B_v),
            np.asarray(W_B_q),
        ],
        axis=1,
    )
    assert Wall.shape == (Hh, NOUT)
    Wt = np.ascontiguousarray(Wall.reshape(KT, 128, NOUT)).astype(BF16)

    inv = 1.0 / (10000.0 ** (np.arange(0, HD, 2, dtype=np.float32) / HD))
    ang = np.arange(S, dtype=np.float32)[:, None] * inv[None, :]
    c, s = np.cos(ang), np.sin(ang)
    cos2 = np.ascontiguousarray(np.concatenate([c, c], axis=1)).astype(BF16)
    sinsw = np.ascontiguousarray(np.concatenate([s, -s], axis=1)).astype(BF16)

    in_maps = []
    for i in range(8):
        tok0 = i * SH
        pos = np.arange(tok0, tok0 + SH) % S
        in_maps.append(
            {
                # pre-transposed (hidden, tokens) so on-chip loads are plain
                "x": np.ascontiguousarray(x2[tok0 : tok0 + SH].T).astype(BF16),
                "w": Wt,
                "cos2": np.ascontiguousarray(cos2[pos]),
                "sinsw": np.ascontiguousarray(sinsw[pos]),
            }
        )
    return in_maps, (B, S)


def assemble_outputs(results, B, S):
    # q rows are (tile, t, h) with token = tile*128 + g*8 + t, cols (g, d)
    qs = []
    for i in range(8):
        a = results[i]["q"].astype(np.float32)
        a = a.reshape(NT, 8, 16, 16, 128).transpose(0, 3, 1, 2, 4)
        qs.append(a.reshape(SH, NH, HD))
    q = np.concatenate(qs, axis=0).reshape(B, S, NH, HD)
    k = np.concatenate(
        [results[i]["kv"][:, 0:2048].astype(np.float32) for i in range(8)],
        axis=0,
    ).reshape(B, S, NH, HD)
    v = np.concatenate(
        [results[i]["kv"][:, 2048:4096].astype(np.float32) for i in range(8)],
        axis=0,
    ).reshape(B, S, NH, HD)
    return q, k, v


def kernel(x, W_A_q, W_B_q, W_A_k, W_B_k, W_A_v, W_B_v):
    from concourse.bass_utils import run_bass_kernel_spmd

    nc = _get_program()
    in_maps, (B, S) = make_in_maps(x, W_A_q, W_B_q, W_A_k, W_B_k, W_A_v, W_B_v)
    res = run_bass_kernel_spmd(nc, in_maps, list(range(8))).results
    return assemble_outputs(res, B, S)


# revision 53
# speedup vs baseline: 1.0604x; 1.0268x over previous
"""Trainium2 Bass kernel for nn_CPLinear (CP-decomposed QKV projection with RoPE).

Computes, for x:(2,4096,2048) and CP-factor weights:
    A_t = x @ W_A_t  (per-token head coefficients),  B_t = x @ W_B_t (shared bases)
    q = einsum('bshr,bsrd->bshd', A_q, rope(B_q)) / 12
    k = A_k * rope(B_k)   (rank-1)
    v = A_v * B_v         (rank-1)

Strategy (8 cores, data-parallel over the 8192 tokens, 1024 tokens/core):
  - All 6 projections fused into one [2048 x 2016] bf16 matmul (PE), with the
    1/12 scale and (h,r)->(r,h) reorder folded into W_A_q host-side.
  - W/x are DMAed k-chunk-interleaved in need-order; tiles 0+1 run a k-outer
    PSUM-resident warmup so the PE starts ~2us into the load.
  - The projection runs as two 2-bank column passes (A/kv block + B_q lo,
    then B_q hi) rotating over three PSUM groups; two banks are dedicated to
    the block-diagonal q matmuls, so no PSUM slot reuse ever stalls the PE.
  - RoPE is applied to [B_q|B_k] as 13 rank slices in 4 wide DVE ops, using
    host-precomputed cos2=[cos,cos] / sinsw=[sin,-sin] tables (rotate-half
    folded into the sign).
  - The per-token rank-12 contraction for q runs on the PE as a block-diagonal
    matmul: 8 tokens/matmul, K=96=(8 tokens x 12 r), M=128=(8 tokens x 16 h),
    N=128=d. Operands are built by a DRAM bounce + per-slot scatter DMAs on
    the sync/gpsimd queues only (engine streams are in-order, so chain-
    dependent DMA issues must never sit on the eviction engine; the fat k/v
    store rides scalar's queue).
  - k/v are per-partition broadcast products (DVE tensor_tensor), fused into
    one store.
  - Outputs are written bf16 in on-chip layout; host reorders q and widens.

Note: this target power-throttles the PE (~29% of the run at 0.5 util), so
wall-clock plateaus near ~188us +-5 regardless of schedule micro-tuning; only
energy-level cuts (bytes moved, MAC-slots, precision) move it further.
"""

import sys

for _p in ("/opt/trn_rl_repo",):
    if _p not in sys.path:
        sys.path.insert(0, _p)

import numpy as np
import ml_dtypes

BF16 = ml_dtypes.bfloat16

SH = 1024          # tokens per core
H = 2048           # hidden
KT = H // 128      # 16 k-tiles
NT = SH // 128     # 8 token tiles per core
NOUT = 2016        # fused projection output width
NH, HD, RQ = 16, 128, 12

_CACHE = {}


def make_nc():
    import concourse.bacc as bacc
    from concourse import mybir

    dt = mybir.dt

    nc = bacc.Bacc(
        "TRN2",
        target_bir_lowering=False,
        debug=False,
        enable_asserts=False,
        num_devices=8,
    )

    x_d = nc.dram_tensor("x", (H, SH), dt.bfloat16, kind="ExternalInput")  # pre-transposed host-side
    w_d = nc.dram_tensor("w", (KT, 128, NOUT), dt.bfloat16, kind="ExternalInput")
    # cos2 = [cos, cos], sinsw = [+sin, -sin] (rotate-half folded into sign)
    cos_d = nc.dram_tensor("cos2", (SH, 128), dt.bfloat16, kind="ExternalInput")
    sin_d = nc.dram_tensor("sinsw", (SH, 128), dt.bfloat16, kind="ExternalInput")
    # q in on-chip layout: row (tile,t,h) = tile*128 + t*16 + h, col g*128+d
    q_d = nc.dram_tensor("q", (SH, NH * HD), dt.bfloat16, kind="ExternalOutput")
    # k and v fused into one store: cols 0:2048 = k, 2048:4096 = v
    kv_d = nc.dram_tensor(
        "kv", (SH, 2 * NH * HD), dt.bfloat16, kind="ExternalOutput"
    )
    return nc, (x_d, w_d, cos_d, sin_d, q_d, kv_d)


def build_body(nc, tc, tensors):
    from contextlib import ExitStack

    from concourse import mybir

    dt = mybir.dt
    x_d, w_d, cos_d, sin_d, q_d, kv_d = tensors

    with ExitStack() as ctx:
        P = ctx.enter_context
        const_pool = P(tc.tile_pool(name="const", bufs=1))
        w_sb = const_pool.tile([128, KT * NOUT], dt.bfloat16, tag="w_sb")
        cos_sb = const_pool.tile([128, NT * 128], dt.bfloat16, tag="cos_sb")
        sin_sb = const_pool.tile([128, NT * 128], dt.bfloat16, tag="sin_sb")
        xT = const_pool.tile([128, KT * SH], dt.bfloat16, tag="xT")
        # combined block-diagonal operand holders: cols 0:2048 = stacked roped
        # B_q (rhs), cols 2048:4096 = block-diag A' (lhsT), per 8-token slot.
        bd_bufs = [
            const_pool.tile([128, 4096], dt.bfloat16, tag=f"bd{i}", name=f"bd{i}")
            for i in range(3)
        ]

        # constant loads: cos/sin first (tiny), then W/x k-chunk interleaved
        # so the k-outer warmup on tiles 0/1 can start after the first chunk.
        w_v = w_sb[:].rearrange("p (k n) -> p k n", k=KT)
        wd_v = w_d[:].rearrange("k p n -> p k n")
        x_v = xT[:].rearrange("p (k t) -> p k t", k=KT)
        xd_v = x_d[:].rearrange("(k p) t -> p k t", p=128)
        # All load pieces are issued in strict need-order, round-robin across
        # the three DMA-capable engines: per-queue service is only a fraction
        # of aggregate HBM bandwidth when several queues are active, so the
        # earliest-needed pieces must not sit behind later ones in any queue.
        # Warmup x covers tokens 0:384 (tiles 0/1/2); the rest of x is first
        # needed by tile 3's projection, much later.
        # each W chunk is striped across the scalar+gpsimd queues (a single
        # queue's service rate is below the warmup's throttled PE demand);
        # cos/sin follow the chunk stream (first needed by tile 0's rope).
        for kk in range(KT):
            sl = slice(kk, kk + 1)
            nc.scalar.dma_start(
                out=w_v[:, sl, 0:1008], in_=wd_v[:, sl, 0:1008]
            )
            nc.gpsimd.dma_start(
                out=w_v[:, sl, 1008:NOUT], in_=wd_v[:, sl, 1008:NOUT]
            )
            nc.sync.dma_start(out=x_v[:, sl, 0:384], in_=xd_v[:, sl, 0:384])
        nc.scalar.dma_start(
            out=cos_sb[:].rearrange("p (t n) -> p t n", t=NT),
            in_=cos_d[:].rearrange("(t p) n -> p t n", p=128),
        )
        nc.gpsimd.dma_start(
            out=sin_sb[:].rearrange("p (t n) -> p t n", t=NT),
            in_=sin_d[:].rearrange("(t p) n -> p t n", p=128),
        )
        for j in range(8):
            sl = slice(2 * j, 2 * j + 2)
            nc.sync.dma_start(out=x_v[:, sl, 384:SH], in_=xd_v[:, sl, 384:SH])
        # zero the A halves once (block-diag zeros outside the scatter
        # positions); the B halves are fully rewritten by the readbacks.
        for tl in bd_bufs:
            nc.gpsimd.memset(tl[0:96, 2048:4096], 0.0)

        # PSUM: one pool of 8 bank-sized [128,512] slots, explicitly
        # assigned. s0..s5 form three 2-bank groups rotated by the two-pass
        # projection (pass A = fused cols 0:992 = [A/kv block | B_q 0:512],
        # pass B = cols 992:2016 = B_q 512:1536); s6/s7 are dedicated to
        # consume's block-diag q matmuls. No slot is ever reused by the next
        # tile before its eviction has long finished.
        ps_pool = P(tc.tile_pool(name="ps", bufs=1, space="PSUM"))
        bq_pool = P(tc.tile_pool(name="bq", bufs=3))
        bqr_pool = P(tc.tile_pool(name="bqr", bufs=3))
        tmp_pool = P(tc.tile_pool(name="tmp", bufs=3))
        small_pool = P(tc.tile_pool(name="small", bufs=4))
        out_pool = P(tc.tile_pool(name="outs", bufs=3))
        dram_pool = P(tc.tile_pool(name="scr", bufs=3, space="DRAM"))

        state = {}

        def ps_tile(slot, nm):
            return ps_pool.tile(
                [128, 512], dt.float32, tag=f"s{slot}", name=nm
            )

        def mm_pass(it, ps2, half, kk, start, stop):
            t0 = it * 128
            lh = xT[:, kk * SH + t0 : kk * SH + t0 + 128]
            wb = kk * NOUT + (0 if half == 0 else 992)
            w0 = 480 if half == 0 else 512
            nc.tensor.matmul(
                ps2[0][:, 0:w0], lh, w_sb[:, wb : wb + w0],
                start=start, stop=stop,
            )
            nc.tensor.matmul(
                ps2[1][:], lh, w_sb[:, wb + w0 : wb + w0 + 512],
                start=start, stop=stop,
            )

        def alloc_tiles(it):
            ak_sb = small_pool.tile([128, 16], dt.bfloat16, tag="ak_sb")
            av_sb = small_pool.tile([128, 16], dt.bfloat16, tag="av_sb")
            bv_sb = small_pool.tile([128, 128], dt.bfloat16, tag="bv_sb")
            # bq_sb holds raw [B_q (12x128) | B_k (128)] = 13 rank slices
            bq_sb = bq_pool.tile([128, 1664], dt.bfloat16, tag="bq_sb")
            # bqr holds roped [B_q|B_k] (0:1664) and A' (1664:1856) so the
            # DRAM bounce is a single DMA
            bqr = bqr_pool.tile([128, 1856], dt.bfloat16, tag="bqr_t")
            return ak_sb, av_sb, bv_sb, bq_sb, bqr

        def evict_A(it, psA, tiles):
            """A/kv-block + first B_q chunk evictions (run during pass B)."""
            ak_sb, av_sb, bv_sb, bq_sb, bqr = tiles
            nc.scalar.copy(bq_sb[:, 0:512], psA[1][:])
            nc.vector.tensor_copy(bq_sb[:, 1536:1664], psA[0][:, 224:352])
            nc.scalar.copy(bqr[:, 1664:1856], psA[0][:, 0:192])
            nc.scalar.copy(ak_sb[:], psA[0][:, 192:208])
            nc.scalar.copy(av_sb[:], psA[0][:, 208:224])
            nc.scalar.copy(bv_sb[:], psA[0][:, 352:480])

        def evict_B_and_post(it, psB, tiles):
            """pass-B evictions, RoPE, bounce + scatter, k/v for tile it."""
            ak_sb, av_sb, bv_sb, bq_sb, bqr = tiles
            t0 = it * 128
            bd = bd_bufs[it % 3]

            nc.scalar.copy(bq_sb[:, 512:1024], psB[0][:])
            nc.vector.tensor_copy(bq_sb[:, 1024:1536], psB[1][:])

            # ---- RoPE on [B_q|B_k] as 13 rank slices (DVE, 4 wide ops) ----
            # t = in*[cos,cos]; u = in*[+sin,-sin];
            # out_lo = t_lo + u_hi; out_hi = t_hi + u_lo
            t_a = tmp_pool.tile([128, 1664], dt.bfloat16, tag="t_a")
            t_b = tmp_pool.tile([128, 1664], dt.bfloat16, tag="t_b")
            R13 = RQ + 1
            cos_t = (
                cos_sb[:, it * 128 : (it + 1) * 128]
                .unsqueeze(1)
                .broadcast_to([128, R13, 128])
            )
            sin_t = (
                sin_sb[:, it * 128 : (it + 1) * 128]
                .unsqueeze(1)
                .broadcast_to([128, R13, 128])
            )
            bqv = bq_sb[:].rearrange("p (r c) -> p r c", r=R13)
            nc.vector.tensor_mul(
                t_a[:].rearrange("p (r c) -> p r c", r=R13), bqv, cos_t
            )
            nc.vector.tensor_mul(
                t_b[:].rearrange("p (r c) -> p r c", r=R13), bqv, sin_t
            )
            tav = t_a[:].rearrange("p (r two d) -> p r two d", r=R13, two=2)
            tbv = t_b[:].rearrange("p (r two d) -> p r two d", r=R13, two=2)
            bqrv = bqr[:, 0:1664].rearrange(
                "p (r two d) -> p r two d", r=R13, two=2
            )
            nc.vector.tensor_add(bqrv[:, :, 0], tav[:, :, 0], tbv[:, :, 1])
            nc.vector.tensor_add(bqrv[:, :, 1], tav[:, :, 1], tbv[:, :, 0])
            bkr_sb = bqr[:, 1536:1664]

            # ---- bounce bqr to DRAM (split across two queues so the
            # chain-critical transfer halves its latency), then scatter back
            scr = dram_pool.tile([128, 1856], dt.bfloat16, tag="scr_b")
            nc.sync.dma_start(out=scr[0:64, :], in_=bqr[0:64, :])
            nc.gpsimd.dma_start(out=scr[64:128, :], in_=bqr[64:128, :])

            # ---- k, v (DVE tensor_mul) + fused output DMA ----
            kvsb = out_pool.tile([128, 4096], dt.bfloat16, tag="kvsb")
            nc.vector.tensor_mul(
                kvsb[:, 0:2048].rearrange("p (h d) -> p h d", h=NH),
                bkr_sb.unsqueeze(1).broadcast_to([128, NH, 128]),
                ak_sb[:].unsqueeze(2).broadcast_to([128, NH, 128]),
            )
            nc.vector.tensor_mul(
                kvsb[:, 2048:4096].rearrange("p (h d) -> p h d", h=NH),
                bv_sb[:].unsqueeze(1).broadcast_to([128, NH, 128]),
                av_sb[:].unsqueeze(2).broadcast_to([128, NH, 128]),
            )
            nc.scalar.dma_start(out=kv_d[t0 : t0 + 128, :], in_=kvsb[:])

            # scatter readbacks: B rhs into bd[:, 0:2048], A lhsT (block-diag)
            # into bd[:, 2048:4096]. Chain-critical, so they live only on the
            # sync/gpsimd queues — scalar's engine stream (evictions) must
            # never block on the bounce, and scalar's HW queue carries the
            # fat kv store.
            sa_v = scr[:, 1664:1856].rearrange(
                "(g t) (r h) -> t r g h", t=8, r=RQ
            )
            sb_v = scr[:, 0:1536].rearrange("(g t) (r d) -> t r g d", t=8, r=RQ)
            l_v = bd[0:96, 2048:4096].rearrange(
                "(t r) (g c) -> t r g c", t=8, g=16
            )
            d_v = bd[0:96, 0:2048].rearrange("(t r) (g d) -> t r g d", t=8, g=16)
            engs = (nc.gpsimd, nc.sync)
            for t in range(8):
                engs[t % 2].dma_start(
                    out=l_v[t][:, :, t * 16 : (t + 1) * 16], in_=sa_v[t]
                )
                engs[(t + 1) % 2].dma_start(out=d_v[t], in_=sb_v[t])

            state[it] = bd

        def consume(it):
            """q contraction (slots s6/s7) + output DMA for tile it."""
            t0 = it * 128
            bd = state.pop(it)

            qsb = out_pool.tile([128, 2048], dt.bfloat16, tag="qsb")
            for gq in range(4):
                qp = ps_tile(6 + gq % 2, f"qp{it}_{gq}")
                for j in range(4):
                    g = gq * 4 + j
                    nc.tensor.matmul(
                        qp[:, j * 128 : (j + 1) * 128],
                        bd[0:96, 2048 + g * 128 : 2048 + (g + 1) * 128],
                        bd[0:96, g * 128 : (g + 1) * 128],
                        start=True,
                        stop=True,
                    )
                if gq % 2 == 0:
                    nc.vector.tensor_copy(
                        qsb[:, gq * 512 : (gq + 1) * 512], qp[:]
                    )
                else:
                    nc.scalar.copy(qsb[:, gq * 512 : (gq + 1) * 512], qp[:])

            # dense on-chip-layout store; host reorders (t,h)(g,d)->(g,t)(h,d).
            # Rides scalar's queue: on sync it would sit ahead of the next
            # tile's chain-critical readbacks and delay them ~4us each.
            nc.scalar.dma_start(out=q_d[t0 : t0 + 128, :], in_=qsb[:])

        # ---- schedule ----
        # warmup: tiles 0 and 1 k-outer across all 8 slots, so the PE starts
        # as soon as the first W/x k-chunk lands and never waits on the load.
        wt0 = alloc_tiles(0)
        wt1 = alloc_tiles(1)
        wA0 = [ps_tile(0, "wA0_0"), ps_tile(1, "wA0_1")]
        wB0 = [ps_tile(2, "wB0_0"), ps_tile(3, "wB0_1")]
        wA1 = [ps_tile(4, "wA1_0"), ps_tile(5, "wA1_1")]
        wB1 = [ps_tile(6, "wB1_0"), ps_tile(7, "wB1_1")]
        for kk in range(KT):
            st, sp = kk == 0, kk == KT - 1
            mm_pass(0, wA0, 0, kk, st, sp)
            mm_pass(0, wB0, 1, kk, st, sp)
            mm_pass(1, wA1, 0, kk, st, sp)
            mm_pass(1, wB1, 1, kk, st, sp)
        evict_A(0, wA0, wt0)
        evict_B_and_post(0, wB0, wt0)
        evict_A(1, wA1, wt1)
        evict_B_and_post(1, wB1, wt1)

        GRP = ((0, 1), (2, 3), (4, 5))

        def produce(it):
            tiles = alloc_tiles(it)
            seq = 2 * (it - 2)
            gA, gB = GRP[seq % 3], GRP[(seq + 1) % 3]
            psA = [ps_tile(gA[0], f"pA{it}_0"), ps_tile(gA[1], f"pA{it}_1")]
            for kk in range(KT):
                mm_pass(it, psA, 0, kk, kk == 0, kk == KT - 1)
            evict_A(it, psA, tiles)
            psB = [ps_tile(gB[0], f"pB{it}_0"), ps_tile(gB[1], f"pB{it}_1")]
            for kk in range(KT):
                mm_pass(it, psB, 1, kk, kk == 0, kk == KT - 1)
            evict_B_and_post(it, psB, tiles)

        # consume(i) is emitted after produce(i+2): its instructions land ~two
        # tiles after its data is ready, so no engine stream ever blocks.
        produce(2)
        consume(0)
        produce(3)
        consume(1)
        produce(4)
        consume(2)
        produce(5)
        consume(3)
        produce(6)
        consume(4)
        produce(7)
        consume(5)
        consume(6)
        consume(7)


def build_program():
    import concourse.tile as tile

    nc, tensors = make_nc()
    with tile.TileContext(nc) as tc:
        build_body(nc, tc, tensors)
    nc.compile()
    return nc


def _get_program():
    if "nc" not in _CACHE:
        _CACHE["nc"] = build_program()
    return _CACHE["nc"]


def make_in_maps(x, W_A_q, W_B_q, W_A_k, W_B_k, W_A_v, W_B_v):
    """Shard + preprocess full inputs into per-core input maps."""
    x = np.asarray(x)
    B, S, Hh = x.shape
    x2 = np.ascontiguousarray(x.reshape(B * S, Hh))

    # fold the 1/RQ scale and the (h,r)->(r,h) column reorder into W_A_q
    WAq = np.asarray(W_A_q).reshape(Hh, NH, RQ).transpose(0, 2, 1).reshape(
        Hh, NH * RQ
    ) / np.float32(RQ)
    Wall = np.concatenate(
        [
            WAq,
            np.asarray(W_A_k),
            np.asarray(W_A_v),
            np.asarray(W_B_k),
            np.asarray(W_B_v),
            np.asarray(W_B_q),
        ],
        axis=1,
    )
    assert Wall.shape == (Hh, NOUT)
    Wt = np.ascontiguousarray(Wall.reshape(KT, 128, NOUT)).astype(BF16)

    inv = 1.0 / (10000.0 ** (np.arange(0, HD, 2, dtype=np.float32) / HD))
    ang = np.arange(S, dtype=np.float32)[:, None] * inv[None, :]
    c, s = np.cos(ang), np.sin(ang)
    cos2 = np.ascontiguousarray(np.concatenate([c, c], axis=1)).astype(BF16)
    sinsw = np.ascontiguousarray(np.concatenate([s, -s], axis=1)).astype(BF16)

    in_maps = []
    for i in range(8):
        tok0 = i * SH
        pos = np.arange(tok0, tok0 + SH) % S
        in_maps.append(
            {
                # pre-transposed (hidden, tokens) so on-chip loads are plain
                "x": np.ascontiguousarray(x2[tok0 : tok0 + SH].T).astype(BF16),
                "w": Wt,
                "cos2": np.ascontiguousarray(cos2[pos]),
                "sinsw": np.ascontiguousarray(sinsw[pos]),
            }
        )
    return in_maps, (B, S)


def assemble_outputs(results, B, S):
    # q rows are (tile, t, h) with token = tile*128 + g*8 + t, cols (g, d)
    qs = []
    for i in range(8):
        a = results[i]["q"].astype(np.float32)
        a = a.reshape(NT, 8, 16, 16, 128).transpose(0, 3, 1, 2, 4)
        qs.append(a.reshape(SH, NH, HD))
    q = np.concatenate(qs, axis=0).reshape(B, S, NH, HD)
    k = np.concatenate(
        [results[i]["kv"][:, 0:2048].astype(np.float32) for i in range(8)],
        axis=0,
    ).reshape(B, S, NH, HD)
    v = np.concatenate(
        [results[i]["kv"][:, 2048:4096].astype(np.float32) for i in range(8)],
        axis=0,
    ).reshape(B, S, NH, HD)
    return q, k, v


def kernel(x, W_A_q, W_B_q, W_A_k, W_B_k, W_A_v, W_B_v):
    from concourse.bass_utils import run_bass_kernel_spmd

    nc = _get_program()
    in_maps, (B, S) = make_in_maps(x, W_A_q, W_B_q, W_A_k, W_B_k, W_A_v, W_B_v)
    res = run_bass_kernel_spmd(nc, in_maps, list(range(8))).results
    return assemble_outputs(res, B, S)
